# revision 1
# baseline (speedup 1.0000x reference)
"""DiffusionGraphConv on 8 Trainium2 NeuronCores (Bass/Tile).

out = sum_k (D^-1 A)^k x W_f[k] + ((D^-1 A)^T)^k x W_b[k] + bias, K=2,
N=50000 nodes, E=800000 edges, B=8, C_in=C_out=64, f32.

Sharding: 8 cores = 4 batch-pairs x 2 diffusion directions (fwd / bwd).
Each core processes its 2 batches packed as 128-f32 node feature rows
(512B gather tokens) and runs the 2 hops of one direction; the host sums
the fwd+bwd partial outputs and adds the bias. No cross-core traffic.

Per hop on device: messages h[src[e]] are fetched with nc.gpsimd.dma_gather
(512B tokens); the scatter-add is a TensorE matmul per 128-edge chunk with a
one-hot matrix S'[t,r] = (r == dst_local[t]) * nv[t] built by one DVE
tensor_scalar(is_equal, mult) op; chunks accumulate per 128-row node block
in PSUM. Each block then contributes h_k @ W[k] to the output accumulator.

Edge streams are grouped by 128-row destination block and split into
lo (src < 32768) / hi runs to satisfy dma_gather's int16 indices. Per-block
chunk counts (Lb, Hb) are the max over the two directions so one SPMD
program serves both; each direction pads its runs to those counts.
"""
import math
import numpy as np

import concourse.bacc as bacc
import concourse.tile as tile
import concourse.mybir as mybir
from concourse.bass_utils import run_bass_kernel_spmd
from concourse.masks import make_identity

P = 128
N_NODES = 50000
N_EDGES = 800000
B, C = 8, 64
NNP = 50048          # nodes padded to a multiple of 128
NB = NNP // P        # 391 row blocks
LO_LIMIT = 32768     # src < LO_LIMIT -> lo gather stream (int16 idx range)
HI_BASE = NNP - 32768  # hi stream gathers from rows [HI_BASE:], idx = src - HI_BASE
GATHER_SLAB = 4096   # tokens per dma_gather instruction
dt = mybir.dt

# pool sizing knobs (tuned against the cost-model timeline)
BUFS = dict(msg_lo=3, msg_hi=3, idxp=8, spp=12, blkp=5, psh=4, pstr=2, psout=2)

_prog_cache = {}


# ---------------- host-side prep ----------------

def _block_counts(dst, src):
    """Per-block (must-lo, must-hi, flexible) source counts.

    src < HI_BASE must use the lo gather base; src >= LO_LIMIT must use hi;
    src in [HI_BASE, LO_LIMIT) is reachable from both bases.
    """
    blk = dst >> 7
    must_lo = np.bincount(blk[src < HI_BASE], minlength=NB)
    must_hi = np.bincount(blk[src >= LO_LIMIT], minlength=NB)
    flex = np.bincount(blk[(src >= HI_BASE) & (src < LO_LIMIT)], minlength=NB)
    return must_lo, must_hi, flex


def _choose_chunks(cf, cb):
    """Shared per-block (Lb, Hb) minimizing Lb+Hb given both directions'
    (must_lo, must_hi, flex) counts, plus each direction's lo-assigned counts."""
    (mlf, mhf, fxf), (mlb, mhb, fxb) = cf, cb
    totf, totb = mlf + mhf + fxf, mlb + mhb + fxb
    Lb = np.zeros(NB, np.int64)
    Hb = np.zeros(NB, np.int64)
    for b in range(NB):
        lmin = (max(mlf[b], mlb[b]) + P - 1) // P
        lmax = min(mlf[b] + fxf[b], mlb[b] + fxb[b]) // P
        best = None
        for L in range(lmin, max(lmin, lmax) + 2):
            rem = max(totf[b] - min(L * P, mlf[b] + fxf[b]),
                      totb[b] - min(L * P, mlb[b] + fxb[b]))
            H = (max(rem, mhf[b], mhb[b]) + P - 1) // P
            if best is None or L + H < best[0] + best[1]:
                best = (L, H)
        Lb[b], Hb[b] = best
    lo_f = np.minimum(Lb * P, mlf + fxf)   # tokens assigned to fwd lo stream
    lo_b = np.minimum(Lb * P, mlb + fxb)
    return Lb, Hb, lo_f - mlf, lo_b - mlb  # flex-to-lo counts per direction


def _build_stream(dst, src, nv, Lb, Hb, flex_to_lo):
    """Padded token streams + chunk-major meta for one direction.

    Block b's lo tokens occupy lo-stream slots [lo_tok_off[b], +Lb[b]*128),
    hi tokens [hi_tok_off[b], +Hb[b]*128). Device chunk c = chunk_off[b]+j
    uses lo chunk lo_chunk_off[b]+j for j < Lb[b], else hi chunk
    hi_chunk_off[b]+j-Lb[b]. Padding tokens: idx 0 / nv 0 / dst-local 0.
    """
    lo_chunk_off = np.concatenate([[0], np.cumsum(Lb)[:-1]])
    hi_chunk_off = np.concatenate([[0], np.cumsum(Hb)[:-1]])
    chunk_off = np.concatenate([[0], np.cumsum(Lb + Hb)[:-1]])
    NCH = int((Lb + Hb).sum())
    TLO, THI = int(Lb.sum()) * P, int(Hb.sum()) * P

    blk = (dst >> 7).astype(np.int64)
    lo = src < HI_BASE
    flex = (src >= HI_BASE) & (src < LO_LIMIT)
    fidx = np.flatnonzero(flex)
    forder = np.argsort(blk[fidx], kind="stable")
    fblk = blk[fidx[forder]]
    fcnt = np.bincount(fblk, minlength=NB)
    fstart = np.concatenate([[0], np.cumsum(fcnt)[:-1]])
    frank = np.arange(fidx.size) - fstart[fblk]
    lo = lo.copy()
    lo[fidx[forder]] = frank < flex_to_lo[fblk]
    assert (np.bincount(blk[lo], minlength=NB) <= Lb * P).all()
    assert (np.bincount(blk[~lo], minlength=NB) <= Hb * P).all()
    order = np.lexsort((~lo, blk))
    d_s, s_s, nv_s = dst[order], src[order], nv[order]
    blk_s, lo_s = blk[order], lo[order]
    gid = blk_s * 2 + (~lo_s).astype(np.int64)
    cnt = np.bincount(gid, minlength=NB * 2)
    gstart = np.concatenate([[0], np.cumsum(cnt)[:-1]])
    rank = np.arange(d_s.size) - gstart[gid]
    lo_tok_off = lo_chunk_off * P
    hi_tok_off = hi_chunk_off * P
    slot = np.where(lo_s, lo_tok_off[blk_s] + rank, hi_tok_off[blk_s] + rank)

    idx_lo = np.zeros(TLO, np.int16)
    nv_lo = np.zeros(TLO, np.float32)
    rm_lo = np.zeros(TLO, np.float32)
    idx_hi = np.zeros(THI, np.int16)
    nv_hi = np.zeros(THI, np.float32)
    rm_hi = np.zeros(THI, np.float32)
    m = lo_s
    idx_lo[slot[m]] = s_s[m].astype(np.int16)
    nv_lo[slot[m]] = nv_s[m]
    rm_lo[slot[m]] = (d_s[m] - (blk_s[m] << 7)).astype(np.float32)
    m = ~lo_s
    idx_hi[slot[m]] = (s_s[m] - HI_BASE).astype(np.int16)
    nv_hi[slot[m]] = nv_s[m]
    rm_hi[slot[m]] = (d_s[m] - (blk_s[m] << 7)).astype(np.float32)

    # chunk-major meta [128, NCH]: column chunk_off[b]+j <- stream chunk
    rowm = np.zeros((P, NCH), np.float32)
    nvm = np.zeros((P, NCH), np.float32)
    # global meta columns of each lo-stream chunk, in stream order
    lo_cols = np.concatenate(
        [chunk_off[b] + np.arange(Lb[b]) for b in range(NB)]) if TLO else []
    hi_cols = np.concatenate(
        [chunk_off[b] + Lb[b] + np.arange(Hb[b]) for b in range(NB)]) if THI else []
    if TLO:
        rowm[:, lo_cols] = rm_lo.reshape(-1, P).T
        nvm[:, lo_cols] = nv_lo.reshape(-1, P).T
    if THI:
        rowm[:, hi_cols] = rm_hi.reshape(-1, P).T
        nvm[:, hi_cols] = nv_hi.reshape(-1, P).T

    def wrap(a):  # [T] -> [128, T/16]; token i at [i%16, i//16], replicated 8x
        return np.ascontiguousarray(np.tile(a.reshape(a.size // 16, 16).T, (8, 1)))

    return wrap(idx_lo), wrap(idx_hi), rowm, nvm


# ---------------- device program (SPMD over the 8 cores) ----------------

def _build_program(Lb, Hb):
    NCH = int((Lb + Hb).sum())
    TLO, THI = int(Lb.sum()) * P, int(Hb.sum()) * P
    nc = bacc.Bacc("TRN2", target_bir_lowering=False, debug=False, num_devices=1)
    x2 = nc.dram_tensor("x2", [NNP, P], dt.float32, kind="ExternalInput")
    w2_d = nc.dram_tensor("w2", [P, 2, P], dt.float32, kind="ExternalInput")
    idx_d = {
        'lo': nc.dram_tensor("idx_lo", [P, TLO // 16], dt.int16, kind="ExternalInput"),
        'hi': nc.dram_tensor("idx_hi", [P, THI // 16], dt.int16, kind="ExternalInput"),
    }
    rowm_d = nc.dram_tensor("rowm", [P, NCH], dt.float32, kind="ExternalInput")
    nvm_d = nc.dram_tensor("nvm", [P, NCH], dt.float32, kind="ExternalInput")
    h1 = nc.dram_tensor("h1", [NNP, P], dt.float32)
    outp = nc.dram_tensor("outp", [NNP, P], dt.float32)
    out2 = nc.dram_tensor("out2", [NNP, P], dt.float32, kind="ExternalOutput")

    with tile.TileContext(nc) as tc:
        with (tc.tile_pool(name="const", bufs=1) as constp,
              tc.tile_pool(name="meta", bufs=1) as metap,
              tc.tile_pool(name="msg_lo", bufs=BUFS["msg_lo"]) as msglop,
              tc.tile_pool(name="msg_hi", bufs=BUFS["msg_hi"]) as msghip,
              tc.tile_pool(name="idxp", bufs=BUFS["idxp"]) as idxp,
              tc.tile_pool(name="spp", bufs=BUFS["spp"]) as spp,
              tc.tile_pool(name="blkp", bufs=BUFS["blkp"]) as blkp,
              tc.tile_pool(name="psh", bufs=BUFS["psh"], space="PSUM") as psum_h,
              tc.tile_pool(name="pstr", bufs=BUFS["pstr"], space="PSUM") as psum_tr,
              tc.tile_pool(name="psout", bufs=BUFS["psout"], space="PSUM") as psum_out):

            iota_i = constp.tile([P, P], dt.int32)
            nc.gpsimd.iota(iota_i[:], pattern=[[1, P]], base=0, channel_multiplier=0)
            iota_f = constp.tile([P, P], dt.float32)
            nc.vector.tensor_copy(iota_f[:], iota_i[:])
            ident = constp.tile([P, P], dt.float32)
            make_identity(nc, ident[:])
            w2_sb = constp.tile([P, 2, P], dt.float32)
            nc.sync.dma_start(out=w2_sb[:], in_=w2_d[:])
            rowm_sb = metap.tile([P, NCH], dt.float32)
            nc.sync.dma_start(out=rowm_sb[:], in_=rowm_d[:])
            nvm_sb = metap.tile([P, NCH], dt.float32)
            nc.sync.dma_start(out=nvm_sb[:], in_=nvm_d[:])

            def hop(src_lo_ap, src_hi_ap, h_out, k, first_hop):
                slab_cache = {'lo': (None, -1), 'hi': (None, -1)}

                def get_chunk(stream, gpos):
                    tile_obj, s_cur = slab_cache[stream]
                    s, j = divmod(gpos, GATHER_SLAB // P)
                    if s != s_cur:
                        T = TLO if stream == 'lo' else THI
                        off = s * GATHER_SLAB
                        g = min(GATHER_SLAB, T - off)
                        it = idxp.tile([P, g // 16], dt.int16, tag="idx")
                        nc.sync.dma_start(
                            out=it[:], in_=idx_d[stream][:, off // 16:(off + g) // 16])
                        pool = msglop if stream == 'lo' else msghip
                        mt = pool.tile([P, g // P, P], dt.float32, tag="m" + stream)
                        nc.gpsimd.dma_gather(
                            out_ap=mt[:],
                            in_ap=src_lo_ap if stream == 'lo' else src_hi_ap,
                            idxs_ap=it[:], num_idxs=g, num_idxs_reg=g,
                            elem_size=P, single_packet=False)
                        slab_cache[stream] = (mt, s)
                        tile_obj = mt
                    return tile_obj[:, j, :]

                c = 0          # global chunk (meta column)
                glo = 0        # lo-stream chunk cursor
                ghi = 0        # hi-stream chunk cursor
                for b in range(NB):
                    L, H = int(Lb[b]), int(Hb[b])
                    CPB = L + H
                    hp = psum_h.tile([P, P], dt.float32, tag="hpsum")
                    for j in range(CPB):
                        if j < L:
                            chunk = get_chunk('lo', glo + j)
                        else:
                            chunk = get_chunk('hi', ghi + (j - L))
                        sp = spp.tile([P, P], dt.float32, tag="sp")
                        nc.vector.tensor_scalar(
                            sp[:], iota_f[:],
                            rowm_sb[:, c + j:c + j + 1], nvm_sb[:, c + j:c + j + 1],
                            mybir.AluOpType.is_equal, mybir.AluOpType.mult)
                        nc.tensor.matmul(hp[:], sp[:], chunk,
                                         start=(j == 0), stop=(j == CPB - 1))
                    c += CPB
                    glo += L
                    ghi += H
                    h_sb = blkp.tile([P, P], dt.float32, tag="h_sb")
                    nc.vector.tensor_copy(h_sb[:], hp[:])
                    if first_hop:
                        nc.sync.dma_start(out=h_out[b * P:(b + 1) * P, :], in_=h_sb[:])
                    tr = psum_tr.tile([P, P], dt.float32, tag="tr")
                    nc.tensor.transpose(tr[:], h_sb[:], ident[:])
                    hT = blkp.tile([P, P], dt.float32, tag="hT")
                    nc.scalar.copy(hT[:], tr[:])
                    op = psum_out.tile([P, P], dt.float32, tag="op")
                    nc.tensor.matmul(op[:], hT[:], w2_sb[:, k, :], start=True, stop=True)
                    ob = blkp.tile([P, P], dt.float32, tag="ob")
                    if first_hop:
                        nc.scalar.copy(ob[:], op[:])
                        nc.sync.dma_start(out=outp[b * P:(b + 1) * P, :], in_=ob[:])
                    else:
                        prev = blkp.tile([P, P], dt.float32, tag="prev")
                        nc.sync.dma_start(out=prev[:], in_=outp[b * P:(b + 1) * P, :])
                        nc.vector.tensor_add(ob[:], prev[:], op[:])
                        nc.sync.dma_start(out=out2[b * P:(b + 1) * P, :], in_=ob[:])

            hop(x2[0:LO_LIMIT, :], x2[HI_BASE:NNP, :], h1, k=0, first_hop=True)
            hop(h1[0:LO_LIMIT, :], h1[HI_BASE:NNP, :], None, k=1, first_hop=False)

    nc.compile()
    return nc


# ---------------- entry point ----------------

def kernel(x, edge_index, edge_vals, W_f, W_b, bias):
    x = np.asarray(x, dtype=np.float32)
    edge_index = np.asarray(edge_index)
    edge_vals = np.asarray(edge_vals, dtype=np.float32)
    W_f = np.asarray(W_f, dtype=np.float32)
    W_b = np.asarray(W_b, dtype=np.float32)
    bias = np.asarray(bias, dtype=np.float32)

    rows = edge_index[0].astype(np.int64)
    cols = edge_index[1].astype(np.int64)
    deg = np.zeros(N_NODES, np.float32)
    np.add.at(deg, rows, edge_vals)
    deg += np.float32(1e-8)
    nv = (edge_vals / deg[rows]).astype(np.float32)

    cf = _block_counts(rows, cols)   # fwd: dst=rows, src=cols
    cb = _block_counts(cols, rows)   # bwd: dst=cols, src=rows
    Lb, Hb, f2l_f, f2l_b = _choose_chunks(cf, cb)

    fwd = _build_stream(rows, cols, nv, Lb, Hb, f2l_f)
    bwd = _build_stream(cols, rows, nv, Lb, Hb, f2l_b)

    key = (Lb.tobytes(), Hb.tobytes())
    if key not in _prog_cache:
        _prog_cache.clear()
        _prog_cache[key] = _build_program(Lb, Hb)
    nc = _prog_cache[key]

    in_maps = []
    for core in range(8):
        pair, d = core >> 1, core & 1
        st = fwd if d == 0 else bwd
        Wd = W_f if d == 0 else W_b
        x2 = np.zeros((NNP, P), np.float32)
        x2[:N_NODES, :C] = x[2 * pair]
        x2[:N_NODES, C:] = x[2 * pair + 1]
        w2 = np.zeros((P, 2, P), np.float32)
        for k in range(2):
            w2[:C, k, :C] = Wd[k]
            w2[C:, k, C:] = Wd[k]
        in_maps.append({"x2": x2, "w2": w2, "idx_lo": st[0], "idx_hi": st[1],
                        "rowm": st[2], "nvm": st[3]})

    results = run_bass_kernel_spmd(nc, in_maps, list(range(8))).results

    out = np.empty((B, N_NODES, C), np.float32)
    for pair in range(4):
        of = results[2 * pair]["out2"][:N_NODES]
        ob = results[2 * pair + 1]["out2"][:N_NODES]
        s = of + ob
        out[2 * pair] = s[:, :C]
        out[2 * pair + 1] = s[:, C:]
    out += bias.reshape(1, 1, C)
    return out



# revision 12
# speedup vs baseline: 2.1341x; 2.1341x over previous
"""DiffusionGraphConv on 8 Trainium2 NeuronCores (Bass/Tile).

out = sum_k (D^-1 A)^k x W_f[k] + ((D^-1 A)^T)^k x W_b[k] + bias, K=2,
N=50000 nodes, E=800000 edges, B=8, C_in=C_out=64, f32.

Sharding: 8 cores = 2 diffusion directions x 2 batch-groups (4 batches
packed per 512B bf16 gather token) x 2 node-halves. No cross-core
traffic: hop 1 processes edges whose DESTINATION block falls in the
core's half (gathering from the replicated x), producing that half of
h1 = (D^-1 A) h0; hop 2 processes edges whose SOURCE falls in the same
half (gathering only from the core's own h1) and scatter-adds into all
destination blocks. The four partial outputs per batch-group (2 dirs x
2 halves) are summed on the host together with the bias.

Per hop on device: messages h[src[e]] are fetched with nc.gpsimd.dma_gather
(512B bf16 tokens); the scatter-add is a TensorE matmul per 128-edge chunk
with a one-hot matrix S[t,n] = (n == dst_local[t]) * nv[t] built by one DVE
tensor_scalar(is_equal, mult) op in bf16 (4x DVE mode); chunks accumulate
per 128-row node block in PSUM. Hop 2 accumulates the transposed block
(lhsT=chunk) directly so no PE transpose is needed before the W matmul.

Blocks are assigned to program positions per-core by descending chunk
count (bin-packing) so one SPMD program's per-position chunk counts,
taken as the max over the 4 edge-shard variants, waste little padding.
"""
import numpy as np
import ml_dtypes

import concourse.bacc as bacc
import concourse.tile as tile
import concourse.mybir as mybir
from concourse.bass_utils import run_bass_kernel_spmd
from concourse.masks import make_identity

P = 128
N_NODES = 50000
N_EDGES = 800000
B, C = 8, 64
NNP = 50048          # nodes padded to a multiple of 128
NB = NNP // P        # 391 destination blocks
NPOS1 = 196          # hop-1 program positions (half0: 196 blocks, half1: 195)
HALF_ROWS = NPOS1 * P   # 25088 rows of h1 per core
LO_LIMIT = 32768     # src < LO_LIMIT -> lo gather stream (int16 idx range)
HI_BASE = NNP - 32768   # hi stream gathers rows [HI_BASE:], idx = src - HI_BASE
GATHER_SLAB = 4096   # tokens per dma_gather instruction
TOKC = 4 * C         # 256 bf16 values per token (4 batches x 64 ch) = 512B
dt = mybir.dt
bf16 = ml_dtypes.bfloat16

BUFS = dict(msg_lo=3, msg_hi=2, msg_lo2=3, idxp=8, spp=12, blkp=5,
            psh=3, pstr=2, psout=2)

_prog_cache = {}


# ---------------- host-side prep ----------------

def _classify(pos, src, npos):
    """Per-position (must-lo, must-hi, flexible) source counts."""
    ml = np.bincount(pos[src < HI_BASE], minlength=npos)
    mh = np.bincount(pos[src >= LO_LIMIT], minlength=npos)
    fx = np.bincount(pos[(src >= HI_BASE) & (src < LO_LIMIT)], minlength=npos)
    return ml, mh, fx


def _choose_chunks(cands, npos):
    """Unified per-position (L, H) chunk counts covering every variant in
    `cands` (list of (ml, mh, fx) triples), minimizing L+H; plus each
    variant's flex-to-lo counts."""
    tots = [ml + mh + fx for (ml, mh, fx) in cands]
    L = np.zeros(npos, np.int64)
    H = np.zeros(npos, np.int64)
    for p in range(npos):
        lmin = max((int(ml[p]) + P - 1) // P for (ml, _, _) in cands)
        lmax = min(int(ml[p] + fx[p]) // P for (ml, _, fx) in cands)
        lfull = max((int(t[p]) + P - 1) // P for t in tots)
        best = None
        for Lc in range(lmin, max(lmin, lmax, lfull) + 2):
            need = 0
            for (ml, mh, fx), tot in zip(cands, tots):
                rem = int(tot[p]) - min(Lc * P, int(ml[p] + fx[p]))
                need = max(need, rem, int(mh[p]))
            Hc = (need + P - 1) // P
            # <= so ties prefer the larger lo stream (keeps hop-2 hi empty)
            if best is None or Lc + Hc <= best[0] + best[1]:
                best = (Lc, Hc)
        L[p], H[p] = best
    L[(L + H) == 0] = 1   # keep every position's PSUM block defined
    f2l = [np.minimum(L * P, ml + fx) - ml for (ml, _, fx) in cands]
    return L, H, f2l


def _build_stream(pos, dstloc, src, nv, L, H, flex_to_lo, npos):
    """Padded token streams + chunk-major meta for one shard variant.

    Position p's lo tokens occupy lo-stream slots [cumsum, +L[p]*128), hi
    tokens likewise. Device chunk c = chunk_off[p]+j uses lo chunk j for
    j < L[p], else hi chunk j-L[p]. Padding tokens: idx 0 / nv 0 / dst 0.
    """
    lo_chunk_off = np.concatenate([[0], np.cumsum(L)[:-1]])
    hi_chunk_off = np.concatenate([[0], np.cumsum(H)[:-1]])
    chunk_off = np.concatenate([[0], np.cumsum(L + H)[:-1]])
    NCH = int((L + H).sum())
    TLO, THI = int(L.sum()) * P, int(H.sum()) * P

    lo = src < HI_BASE
    flex = (src >= HI_BASE) & (src < LO_LIMIT)
    fidx = np.flatnonzero(flex)
    forder = np.argsort(pos[fidx], kind="stable")
    fpos = pos[fidx[forder]]
    fcnt = np.bincount(fpos, minlength=npos)
    fstart = np.concatenate([[0], np.cumsum(fcnt)[:-1]])
    frank = np.arange(fidx.size) - fstart[fpos]
    lo = lo.copy()
    lo[fidx[forder]] = frank < flex_to_lo[fpos]
    assert (np.bincount(pos[lo], minlength=npos) <= L * P).all()
    assert (np.bincount(pos[~lo], minlength=npos) <= H * P).all()
    order = np.lexsort((~lo, pos))
    dl_s, s_s, nv_s = dstloc[order], src[order], nv[order]
    pos_s, lo_s = pos[order], lo[order]
    gid = pos_s * 2 + (~lo_s).astype(np.int64)
    cnt = np.bincount(gid, minlength=npos * 2)
    gstart = np.concatenate([[0], np.cumsum(cnt)[:-1]])
    rank = np.arange(dl_s.size) - gstart[gid]
    slot = np.where(lo_s, lo_chunk_off[pos_s] * P + rank,
                    hi_chunk_off[pos_s] * P + rank)

    idx_lo = np.zeros(TLO, np.int16)
    nv_lo = np.zeros(TLO, np.float32)
    rm_lo = np.zeros(TLO, np.float32)
    idx_hi = np.zeros(THI, np.int16)
    nv_hi = np.zeros(THI, np.float32)
    rm_hi = np.zeros(THI, np.float32)
    m = lo_s
    idx_lo[slot[m]] = s_s[m].astype(np.int16)
    nv_lo[slot[m]] = nv_s[m]
    rm_lo[slot[m]] = dl_s[m].astype(np.float32)
    m = ~lo_s
    idx_hi[slot[m]] = (s_s[m] - HI_BASE).astype(np.int16)
    nv_hi[slot[m]] = nv_s[m]
    rm_hi[slot[m]] = dl_s[m].astype(np.float32)

    rowm = np.zeros((P, NCH), np.float32)
    nvm = np.zeros((P, NCH), np.float32)
    lo_cols = (np.repeat(chunk_off, L) +
               (np.arange(TLO // P) - np.repeat(lo_chunk_off, L))) if TLO else []
    hi_cols = (np.repeat(chunk_off + L, H) +
               (np.arange(THI // P) - np.repeat(hi_chunk_off, H))) if THI else []
    if TLO:
        rowm[:, lo_cols] = rm_lo.reshape(-1, P).T
        nvm[:, lo_cols] = nv_lo.reshape(-1, P).T
    if THI:
        rowm[:, hi_cols] = rm_hi.reshape(-1, P).T
        nvm[:, hi_cols] = nv_hi.reshape(-1, P).T

    def wrap(a):  # [T] -> [128, T/16]; token i at [i%16, i//16], replicated 8x
        if a.size == 0:
            return np.zeros((P, 0), np.int16)
        return np.ascontiguousarray(np.tile(a.reshape(a.size // 16, 16).T, (8, 1)))

    return wrap(idx_lo), wrap(idx_hi), rowm, nvm


def _pack_positions(tot, blocks, npos):
    """Assign `blocks` to program positions by descending token count.
    Returns (order, inv) where order[p] = absolute block (-1 pad) and
    inv[blk] = position."""
    o = blocks[np.argsort(-tot[blocks], kind="stable")]
    order = np.full(npos, -1, np.int64)
    order[:o.size] = o
    inv = np.full(NB, -1, np.int64)
    inv[o] = np.arange(o.size)
    return order, inv


# ---------------- device program (SPMD over the 8 cores) ----------------

def _build_program(L1, H1, L2):
    NCH1 = int((L1 + H1).sum())
    NCH2 = int(L2.sum())
    TLO1, THI1 = int(L1.sum()) * P, int(H1.sum()) * P
    T2 = NCH2 * P
    nc = bacc.Bacc("TRN2", target_bir_lowering=False, debug=False, num_devices=1)
    x2 = nc.dram_tensor("x2", [NNP, TOKC], dt.bfloat16, kind="ExternalInput")
    w2_d = nc.dram_tensor("w2", [P, 2, P], dt.bfloat16, kind="ExternalInput")
    idx_d = {
        'lo': nc.dram_tensor("idx_lo", [P, TLO1 // 16], dt.int16, kind="ExternalInput"),
        'hi': nc.dram_tensor("idx_hi", [P, THI1 // 16], dt.int16, kind="ExternalInput"),
        'lo2': nc.dram_tensor("idx2", [P, T2 // 16], dt.int16, kind="ExternalInput"),
    }
    rowm1_d = nc.dram_tensor("rowm1", [P, NCH1], dt.float32, kind="ExternalInput")
    nvm1_d = nc.dram_tensor("nvm1", [P, NCH1], dt.float32, kind="ExternalInput")
    rowm2_d = nc.dram_tensor("rowm2", [P, NCH2], dt.float32, kind="ExternalInput")
    nvm2_d = nc.dram_tensor("nvm2", [P, NCH2], dt.float32, kind="ExternalInput")
    h1 = nc.dram_tensor("h1", [HALF_ROWS, TOKC], dt.bfloat16)
    outA = nc.dram_tensor("outA", [HALF_ROWS, TOKC], dt.bfloat16, kind="ExternalOutput")
    outB = nc.dram_tensor("outB", [NNP, TOKC], dt.bfloat16, kind="ExternalOutput")

    with tile.TileContext(nc) as tc:
        with (tc.tile_pool(name="const", bufs=1) as constp,
              tc.tile_pool(name="meta", bufs=1) as metap,
              tc.tile_pool(name="msg_lo", bufs=BUFS["msg_lo"]) as msglop,
              tc.tile_pool(name="msg_hi", bufs=BUFS["msg_hi"]) as msghip,
              tc.tile_pool(name="msg_lo2", bufs=BUFS["msg_lo2"]) as msglo2p,
              tc.tile_pool(name="idxp", bufs=BUFS["idxp"]) as idxp,
              tc.tile_pool(name="spp", bufs=BUFS["spp"]) as spp,
              tc.tile_pool(name="blkp", bufs=BUFS["blkp"]) as blkp,
              tc.tile_pool(name="psh", bufs=BUFS["psh"], space="PSUM") as psum_h,
              tc.tile_pool(name="pstr", bufs=BUFS["pstr"], space="PSUM") as psum_tr,
              tc.tile_pool(name="psout", bufs=BUFS["psout"], space="PSUM") as psum_out):

            iota_i = constp.tile([P, P], dt.int32)
            nc.gpsimd.iota(iota_i[:], pattern=[[1, P]], base=0, channel_multiplier=0)
            iota_f = constp.tile([P, P], dt.bfloat16)
            nc.vector.tensor_copy(iota_f[:], iota_i[:])
            ident = constp.tile([P, P], dt.bfloat16)
            make_identity(nc, ident[:])
            w2_sb = constp.tile([P, 2, P], dt.bfloat16)
            nc.sync.dma_start(out=w2_sb[:], in_=w2_d[:])
            rowm1_sb = metap.tile([P, NCH1], dt.float32)
            nc.sync.dma_start(out=rowm1_sb[:], in_=rowm1_d[:])
            nvm1_sb = metap.tile([P, NCH1], dt.float32)
            nc.sync.dma_start(out=nvm1_sb[:], in_=nvm1_d[:])
            rowm2_sb = metap.tile([P, NCH2], dt.float32)
            nc.sync.dma_start(out=rowm2_sb[:], in_=rowm2_d[:])
            nvm2_sb = metap.tile([P, NCH2], dt.float32)
            nc.sync.dma_start(out=nvm2_sb[:], in_=nvm2_d[:])

            slab_cache = {}

            def get_chunk(stream, src_ap, pool, T, gpos):
                tile_obj, s_cur = slab_cache.get(stream, (None, -1))
                s, j = divmod(gpos, GATHER_SLAB // P)
                if s != s_cur:
                    off = s * GATHER_SLAB
                    g = min(GATHER_SLAB, T - off)
                    it = idxp.tile([P, g // 16], dt.int16, tag="idx")
                    nc.sync.dma_start(
                        out=it[:], in_=idx_d[stream][:, off // 16:(off + g) // 16])
                    mt = pool.tile([P, g // P, TOKC], dt.bfloat16, tag="m" + stream)
                    nc.gpsimd.dma_gather(
                        out_ap=mt[:], in_ap=src_ap,
                        idxs_ap=it[:], num_idxs=g, num_idxs_reg=g,
                        elem_size=TOKC, single_packet=False)
                    slab_cache[stream] = (mt, s)
                    tile_obj = mt
                return tile_obj, j

            def build_sp(rowm_sb, nvm_sb, c):
                sp = spp.tile([P, P], dt.bfloat16, tag="sp")
                nc.vector.tensor_scalar(
                    sp[:], iota_f[:],
                    rowm_sb[:, c:c + 1], nvm_sb[:, c:c + 1],
                    mybir.AluOpType.is_equal, mybir.AluOpType.mult)
                return sp

            # ---- hop 1: h1[half] = (D^-1 A) h0, out += h1 @ W[0] ----
            c = 0
            glo = 0
            ghi = 0
            for p in range(NPOS1):
                Lp, Hp = int(L1[p]), int(H1[p])
                CPB = Lp + Hp
                hp = psum_h.tile([P, 2, P], dt.float32, tag="hp")
                for j in range(CPB):
                    if j < Lp:
                        mt, jj = get_chunk('lo', x2[0:LO_LIMIT, :], msglop,
                                           TLO1, glo + j)
                    else:
                        mt, jj = get_chunk('hi', x2[HI_BASE:NNP, :], msghip,
                                           THI1, ghi + (j - Lp))
                    sp = build_sp(rowm1_sb, nvm1_sb, c + j)
                    nc.tensor.matmul(hp[:], sp[:], mt[:, jj, :],
                                     start=(j == 0), stop=(j == CPB - 1))
                c += CPB
                glo += Lp
                ghi += Hp
                h_sb = blkp.tile([P, 2, P], dt.bfloat16, tag="h_sb")
                nc.scalar.copy(h_sb[:], hp[:])
                nc.sync.dma_start(out=h1[p * P:(p + 1) * P, :], in_=h_sb[:])
                tr = psum_tr.tile([P, 2, P], dt.bfloat16, tag="tr")
                nc.tensor.transpose(tr[:, 0, :], h_sb[:, 0, :], ident[:])
                nc.tensor.transpose(tr[:, 1, :], h_sb[:, 1, :], ident[:])
                trs = blkp.tile([P, 2, P], dt.bfloat16, tag="trs")
                nc.vector.tensor_copy(trs[:], tr[:])
                op = psum_out.tile([P, 2, P], dt.float32, tag="op")
                nc.tensor.matmul(op[:, 0, :], trs[:, 0, :], w2_sb[:, 0, :],
                                 start=True, stop=True)
                nc.tensor.matmul(op[:, 1, :], trs[:, 1, :], w2_sb[:, 0, :],
                                 start=True, stop=True)
                ob = blkp.tile([P, 2, P], dt.bfloat16, tag="ob")
                nc.scalar.copy(ob[:], op[:])
                nc.sync.dma_start(out=outA[p * P:(p + 1) * P, :], in_=ob[:])

            # ---- hop 2: h2 = (D^-1 A)|src-half h1, out += h2 @ W[1] ----
            # Accumulated transposed (lhsT=chunk half) so the block is already
            # feature-major for the W matmul.
            # The barrier orders hop-2's h1 gathers after hop-1's h1 writes
            # (DRAM RAW is not tracked at tile granularity).
            tc.strict_bb_all_engine_barrier()
            c = 0
            for p in range(NB):
                CPB = int(L2[p])
                hp = psum_h.tile([P, 2, P], dt.float32, tag="hp")
                for j in range(CPB):
                    mt, jj = get_chunk('lo2', h1[0:HALF_ROWS, :], msglo2p, T2,
                                       c + j)
                    sp = build_sp(rowm2_sb, nvm2_sb, c + j)
                    nc.tensor.matmul(hp[:], sp[:], mt[:, jj, :],
                                     start=(j == 0), stop=(j == CPB - 1))
                c += CPB
                h_sb = blkp.tile([P, 2, P], dt.bfloat16, tag="h_sb")
                nc.scalar.copy(h_sb[:], hp[:])
                tr = psum_tr.tile([P, 2, P], dt.bfloat16, tag="tr")
                nc.tensor.transpose(tr[:, 0, :], h_sb[:, 0, :], ident[:])
                nc.tensor.transpose(tr[:, 1, :], h_sb[:, 1, :], ident[:])
                trs = blkp.tile([P, 2, P], dt.bfloat16, tag="trs")
                nc.vector.tensor_copy(trs[:], tr[:])
                op = psum_out.tile([P, 2, P], dt.float32, tag="op")
                nc.tensor.matmul(op[:, 0, :], trs[:, 0, :], w2_sb[:, 1, :],
                                 start=True, stop=True)
                nc.tensor.matmul(op[:, 1, :], trs[:, 1, :], w2_sb[:, 1, :],
                                 start=True, stop=True)
                ob = blkp.tile([P, 2, P], dt.bfloat16, tag="ob")
                nc.scalar.copy(ob[:], op[:])
                nc.sync.dma_start(out=outB[p * P:(p + 1) * P, :], in_=ob[:])

    nc.compile()
    return nc


# ---------------- entry point ----------------

def kernel(x, edge_index, edge_vals, W_f, W_b, bias):
    x = np.asarray(x, dtype=np.float32)
    edge_index = np.asarray(edge_index)
    edge_vals = np.asarray(edge_vals, dtype=np.float32)
    W_f = np.asarray(W_f, dtype=np.float32)
    W_b = np.asarray(W_b, dtype=np.float32)
    bias = np.asarray(bias, dtype=np.float32)

    rows = edge_index[0].astype(np.int64)
    cols = edge_index[1].astype(np.int64)
    deg = np.zeros(N_NODES, np.float32)
    np.add.at(deg, rows, edge_vals)
    deg += np.float32(1e-8)
    nv = (edge_vals / deg[rows]).astype(np.float32)

    halves = [np.arange(0, NPOS1), np.arange(NPOS1, NB)]
    v1 = []   # hop-1 variants: (pos, dstloc, src, nv, order)
    v2 = []   # hop-2 variants: (pos, dstloc, srcloc, nv, order, inv1)
    for d in range(2):
        dst, src = (rows, cols) if d == 0 else (cols, rows)
        dblk = dst >> 7
        dloc = dst & (P - 1)
        sblk = src >> 7
        tot1 = np.bincount(dblk, minlength=NB)
        tot2 = np.bincount(dblk, weights=(sblk >= NPOS1).astype(np.float64),
                           minlength=NB)
        for h in range(2):
            sel = (dblk >= NPOS1) == (h == 1)
            order1, inv1 = _pack_positions(tot1, halves[h], NPOS1)
            v1.append((inv1[dblk[sel]], dloc[sel], src[sel], nv[sel], order1))
            sel2 = (sblk >= NPOS1) == (h == 1)
            t2 = tot2 if h == 1 else (tot1 - tot2)
            order2, inv2 = _pack_positions(t2, np.arange(NB), NB)
            srcloc = inv1[sblk[sel2]] * P + (src[sel2] & (P - 1))
            v2.append((inv2[dblk[sel2]], dloc[sel2], srcloc, nv[sel2],
                       order2, inv1))

    c1 = [_classify(pos, src, NPOS1) for (pos, _, src, _, _) in v1]
    L1, H1, f2l1 = _choose_chunks(c1, NPOS1)
    c2 = [_classify(pos, src, NB) for (pos, _, src, _, _, _) in v2]
    L2, H2, f2l2 = _choose_chunks(c2, NB)
    assert H2.sum() == 0, "hop-2 sources must fit the lo stream"

    s1 = [_build_stream(pos, dl, src, nvv, L1, H1, f2l1[i], NPOS1)
          for i, (pos, dl, src, nvv, _) in enumerate(v1)]
    s2 = [_build_stream(pos, dl, src, nvv, L2, H2, f2l2[i], NB)
          for i, (pos, dl, src, nvv, _, _) in enumerate(v2)]

    key = (L1.tobytes(), H1.tobytes(), L2.tobytes())
    if key not in _prog_cache:
        _prog_cache.clear()
        _prog_cache[key] = _build_program(L1, H1, L2)
    nc = _prog_cache[key]

    in_maps = []
    for core in range(8):
        d, g, h = core >> 2, (core >> 1) & 1, core & 1
        vi = d * 2 + h
        Wd = W_f if d == 0 else W_b
        x2 = np.zeros((NNP, TOKC), bf16)
        x2[:N_NODES] = x[4 * g:4 * g + 4].transpose(1, 0, 2).reshape(
            N_NODES, TOKC).astype(bf16)
        w2 = np.zeros((P, 2, P), bf16)
        for k in range(2):
            for a in range(2):
                w2[C * a:C * a + C, k, C * a:C * a + C] = Wd[k].astype(bf16)
        in_maps.append({
            "x2": x2, "w2": w2,
            "idx_lo": s1[vi][0], "idx_hi": s1[vi][1],
            "rowm1": s1[vi][2], "nvm1": s1[vi][3],
            "idx2": s2[vi][0],
            "rowm2": s2[vi][2], "nvm2": s2[vi][3],
        })

    results = run_bass_kernel_spmd(nc, in_maps, list(range(8))).results

    out = np.empty((B, N_NODES, C), np.float32)
    for g in range(2):
        acc = np.zeros((NNP, TOKC), np.float32)
        for d in range(2):
            for h in range(2):
                vi = d * 2 + h
                r = results[(d << 2) | (g << 1) | h]
                order2 = v2[vi][4]
                inv2 = np.argsort(order2)
                acc += np.asarray(r["outB"]).astype(np.float32).reshape(
                    NB, P, TOKC)[inv2].reshape(NNP, TOKC)
                order1 = v1[vi][4]
                nreal = halves[h].size
                oa = np.asarray(r["outA"]).astype(np.float32).reshape(
                    NPOS1, P, TOKC)[:nreal]
                accb = acc.reshape(NB, P, TOKC)
                accb[order1[:nreal]] += oa
        for bl in range(4):
            out[4 * g + bl] = acc[:N_NODES, C * bl:C * bl + C]
    out += bias.reshape(1, 1, C)
    return out
